# revision 7
# baseline (speedup 1.0000x reference)
"""LightGCN 3-layer propagation + CVIB loss on 8 Trainium2 NeuronCores.

Self-contained kernel: takes full (unsharded) numpy inputs, row-shards the
graph across 8 cores with a slice-major table layout, runs a Bass/Tile SPMD
program and combines per-core partial sums on the host into the two losses.

v2 design:
  - Table rows laid out slice-major: 5 slices x (8 cores x nt_s tiles x 128).
    Each slice is its own Shared DRAM tensor, AllGathered independently so
    slice AGs overlap with compute of later slices (per-tensor deps).
  - fp8 on the wire (stage + AllGather), upconverted to a local bf16 copy
    per slice via a single SWDGE cast-DMA (DRAM->DRAM); gathers read bf16
    (dma_gather needs 256B granules so fp8 cannot be gathered directly).
  - No acc round-trip: layers 0/1 just emit stage; the last layer folds
    own0/table1/table2 own-rows into PSUM via identity matmuls.
  - gather + one-hot-matmul segment-sum SpMM per layer; data-parallel loss.
"""
import sys

sys.path.insert(0, "/opt/trn_rl_repo")

import numpy as np
import ml_dtypes

import concourse.bass as bass
import concourse.bacc as bacc
import concourse.tile as tile
from concourse import mybir
from concourse.bass_utils import run_bass_kernel_spmd

# ---------------- problem constants ----------------
N_USERS = 100000
N_ITEMS = 40000
N_NODES = N_USERS + N_ITEMS
EMB = 128
BATCH = 8192
N_LAYERS = 3
ALPHA = 0.1
GAMMA = 0.01

# ---------------- sharding config ----------------
P = 128
NCORES = 8
TILES = 137                      # row tiles per core
RPC = TILES * P                  # rows per core = 17536
NT = NCORES * RPC                # padded table rows = 140288
NSLICE = 5
T_BOUNDS = [0, 28, 56, 83, 110, 137]          # tile ranges per slice
NT_S = [28, 28, 27, 27, 27]                   # tiles per core per slice
SLICE_ROWS = [NCORES * n * P for n in NT_S]   # 28672/28672/27648/27648/27648
CB = np.concatenate([[0], np.cumsum(SLICE_ROWS)]).astype(np.int64)
NCHUNK = NSLICE                  # gather source windows == slices

# tunables
import os as _os
SB_T = int(_os.environ.get("BK_SB_T", "4"))   # tiles per gather superblock
DEBUG_SB_LIMIT = (None if "BK_SB_LIMIT" not in _os.environ
                  else int(_os.environ["BK_SB_LIMIT"]))
DEBUG_SKIP_LOSS = bool(int(_os.environ.get("BK_SKIP_LOSS", "0")))
DEBUG_LAYERS = int(_os.environ.get("BK_LAYERS", str(N_LAYERS)))
GMAX = int(_os.environ.get("BK_GMAX", "1024"))  # idxs per dma_gather call
DEBUG_SINGLE = False             # 1-core, collectives replaced by local copies
DEBUG_SKIP_PERSIST = bool(int(_os.environ.get("BK_SKIP_PERSIST", "0")))
DEBUG_GATHER_ONLY = False        # only dma_gathers (no S/matmul/stores)
DEBUG_COMPUTE_ONLY = False       # skip dma_gathers, keep everything else

DT = mybir.dt.bfloat16           # gather/matmul dtype (local tables)
NPDT = ml_dtypes.bfloat16
WDT = mybir.dt.float8e4          # wire dtype (stage + AllGather tables)
NPWDT = ml_dtypes.float8_e4m3


def cdiv(a, b):
    return (a + b - 1) // b


def _slice_of_tile(t):
    for s in range(NSLICE):
        if t < T_BOUNDS[s + 1]:
            return s
    raise ValueError(t)


_TILE_SLICE = np.array([_slice_of_tile(t) for t in range(TILES)], np.int64)


def row_of(core, tile_, lane):
    """Slice-major table row for (core, local tile, lane). Vectorized."""
    s = _TILE_SLICE[tile_]
    nt = np.asarray(NT_S, np.int64)[s]
    t0 = np.asarray(T_BOUNDS, np.int64)[s]
    return CB[s] + core * nt * P + (tile_ - t0) * P + lane


def row_to_ctl(row):
    """Inverse of row_of. Vectorized: row -> (core, tile, lane)."""
    row = np.asarray(row, np.int64)
    s = np.searchsorted(CB, row, side="right") - 1
    nt = np.asarray(NT_S, np.int64)[s]
    t0 = np.asarray(T_BOUNDS, np.int64)[s]
    r = row - CB[s]
    core = r // (nt * P)
    rr = r % (nt * P)
    return core, t0 + rr // P, rr % P


def balance_core(node_ids, cnt, n_tiles, P=128, NCHUNK=5):
    """node_ids: nodes currently in one (core, slice) segment of n_tiles
    tiles. cnt: [n_nodes, NCHUNK] dst-edge chunk counts. Returns new order
    (len = n_tiles*P) flattening per-(tile, chunk) maxima."""
    n = len(node_ids)
    assert n == n_tiles * P
    tot = cnt.sum(1)
    order = np.argsort(-tot, kind="stable")
    tile_of = np.empty(n, np.int64)
    tile_of[order] = np.arange(n) % n_tiles
    sums = np.zeros((n_tiles, NCHUNK), np.int64)
    for c in range(NCHUNK):
        np.add.at(sums[:, c], tile_of, cnt[:, c])

    cap = cdiv(int(sums.sum(0).max() // n_tiles) + 1, P) * P

    for it in range(4):
        over = np.argwhere(sums > cap)
        if len(over) == 0:
            break
        for t, c in over:
            idxs_t = np.nonzero(tile_of == t)[0]
            if sums[t, c] <= cap:
                continue
            byc = idxs_t[np.argsort(-cnt[idxs_t, c], kind="stable")]
            for nid in byc:
                if sums[t, c] <= cap:
                    break
                t2 = int(np.argmin(sums[:, c] + (np.arange(n_tiles) == t) * 10**9))
                idxs_t2 = np.nonzero(tile_of == t2)[0]
                nid2 = idxs_t2[np.argmin(cnt[idxs_t2, c])]
                d = cnt[nid] - cnt[nid2]
                if sums[t2, c] + d[c] > cap:
                    continue
                sums[t] -= d
                sums[t2] += d
                tile_of[nid] = t2
                tile_of[nid2] = t
    new_order = np.argsort(tile_of, kind="stable")
    return node_ids[new_order]


# ============================================================
# Host-side packing
# ============================================================

def _wrap_idx(lst):
    """int16 list (len % 16 == 0) -> wrapped [16, len/16] block."""
    return lst.reshape(-1, 16).T


class Plan:
    """Static (data-shape) schedule shared by all cores."""
    pass


def host_pack(user_emb, item_emb, graph_vals, edge_user, edge_item,
              users, pos_items, neg_items, sampled_user, sampled_items):
    rows0 = np.concatenate([edge_user, edge_item + N_USERS]).astype(np.int64)
    cols0 = np.concatenate([edge_item + N_USERS, edge_user]).astype(np.int64)
    vals = np.asarray(graph_vals, np.float32)

    # node -> table-row permutation: deal nodes by degree across cores,
    # then within-core degree ranks across the 137 tiles (slice-major rows).
    deg = np.bincount(rows0, minlength=N_NODES)
    order = np.argsort(-deg, kind="stable")
    k = np.arange(N_NODES)
    q = k // NCORES
    perm = np.empty(N_NODES, np.int64)
    perm[order] = row_of(k % NCORES, q % TILES, q // TILES)

    # within-(core, slice) rebalance of tiles to flatten per-(tile, chunk)
    # dst-edge counts. Chunk of every node is invariant (segments are whole
    # slices), so col chunks don't move.
    cseg = np.searchsorted(CB, perm[cols0], side="right") - 1
    n_holes = NT - N_NODES
    cnt = np.zeros((N_NODES + n_holes, NCHUNK), np.int64)
    np.add.at(cnt, (rows0, cseg), 1)
    inv = np.full(NT, -1, np.int64)     # table row -> node (or virtual hole)
    inv[perm] = np.arange(N_NODES)
    hole_rows = np.nonzero(inv < 0)[0]
    inv[hole_rows] = N_NODES + np.arange(n_holes)
    perm_ext = np.empty(N_NODES + n_holes, np.int64)
    perm_ext[:N_NODES] = perm
    perm_ext[N_NODES:] = hole_rows
    for i in range(NCORES):
        for s in range(NSLICE):
            lo = CB[s] + i * NT_S[s] * P
            rows_seg = np.arange(lo, lo + NT_S[s] * P)
            nodes = inv[rows_seg]
            new_nodes = balance_core(nodes, cnt[nodes], NT_S[s],
                                     P=P, NCHUNK=NCHUNK)
            perm_ext[new_nodes] = rows_seg
    perm = perm_ext[:N_NODES]
    rows = perm[rows0]
    cols = perm[cols0]
    assert (np.searchsorted(CB, cols, side="right") - 1 == cseg).all()

    core_of, tile_of, lrow_i = row_to_ctl(rows)
    lrow = lrow_i.astype(np.float32)
    chunk = cseg
    lcol = (cols - CB[chunk]).astype(np.int16)

    # group edges by (core, tile, chunk)
    key = (core_of * TILES + tile_of) * NCHUNK + chunk
    order = np.argsort(key, kind="stable")
    key_s = key[order]
    lcol_s = lcol[order]
    lrow_s = lrow[order]
    vals_s = vals[order]
    ncell = NCORES * TILES * NCHUNK
    counts = np.bincount(key_s, minlength=ncell).reshape(NCORES, TILES, NCHUNK)
    starts = np.zeros(ncell + 1, np.int64)
    np.cumsum(counts.reshape(-1), out=starts[1:])

    # static slot budgets: max over cores
    B = cdiv(counts, P).max(axis=0)          # [TILES, NCHUNK] slots
    n_sb = cdiv(TILES, SB_T)
    sb_tiles = [list(range(s * SB_T, min((s + 1) * SB_T, TILES)))
                for s in range(n_sb)]

    plan = Plan()
    plan.B = B
    plan.sb_tiles = sb_tiles
    slot_off = np.zeros((TILES, NCHUNK), np.int64)
    gathers = []   # (sb, c, idx_col, n_idx, dst_slot)
    sb_slot_base = []
    sb_nslots = []
    g_off = 0
    idx_cols = 0
    for s, tl in enumerate(sb_tiles):
        sb_slot_base.append(g_off)
        sb_start = g_off
        for c in range(NCHUNK):
            n_slots = int(B[tl, c].sum())
            if n_slots == 0:
                continue
            n_idx = n_slots * P
            gathers.append(dict(sb=s, c=c, idx_col=idx_cols, n_idx=n_idx,
                                dst_slot=g_off - sb_start))
            off = g_off
            for t in tl:
                slot_off[t, c] = off
                off += int(B[t, c])
            idx_cols += n_idx // 16
            g_off += n_slots
        sb_nslots.append(g_off - sb_start)
    plan.slot_off = slot_off
    plan.gathers = gathers
    plan.sb_slot_base = sb_slot_base
    plan.sb_nslots = sb_nslots
    plan.tot_slots = g_off
    plan.g_slots = max(sb_nslots)
    plan.idx_cols_edges = idx_cols

    # ---- batch (loss) packing: ONE union stream sorted by (chunk_a, chunk_b);
    # per-lane set-membership masks (3) recover the per-set sums ----
    users = perm[np.asarray(users, np.int64)]
    pos_t = perm[np.asarray(pos_items, np.int64) + N_USERS]
    neg_t = perm[np.asarray(neg_items, np.int64) + N_USERS]
    su = perm[np.asarray(sampled_user, np.int64)]
    si = perm[np.asarray(sampled_items, np.int64) + N_USERS]

    bpc = BATCH // NCORES          # 1024 per core
    spc = 2 * BATCH // NCORES      # 2048 per core

    users_r = users.reshape(NCORES, bpc)
    pos_r = pos_t.reshape(NCORES, bpc)
    neg_r = neg_t.reshape(NCORES, bpc)
    su_r = su.reshape(NCORES, spc)
    si_r = si.reshape(NCORES, spc)
    a_all = np.concatenate([users_r, users_r, su_r], axis=1)  # [NCORES, 4096]
    b_all = np.concatenate([pos_r, neg_r, si_r], axis=1)
    sid = np.concatenate([np.zeros(bpc, np.int64), np.ones(bpc, np.int64),
                          np.full(spc, 2, np.int64)])

    def pack_union():
        ca = (np.searchsorted(CB, a_all.reshape(-1), side="right") - 1
              ).reshape(a_all.shape)
        cb = (np.searchsorted(CB, b_all.reshape(-1), side="right") - 1
              ).reshape(b_all.shape)
        cell = ca * NCHUNK + cb
        counts = np.stack([np.bincount(cell[i], minlength=NCHUNK * NCHUNK)
                           for i in range(NCORES)])     # [NCORES, 25]
        cellB = cdiv(counts, P).max(axis=0)             # slots per cell
        per_core = []
        for i in range(NCORES):
            o = np.argsort(cell[i], kind="stable")
            a_s, b_s, sid_s = a_all[i][o], b_all[i][o], sid[o]
            a_out, b_out, m_out = [], [], []
            st = 0
            for cc in range(NCHUNK * NCHUNK):
                n = int(counts[i, cc])
                want = int(cellB[cc]) * P
                if want == 0:
                    assert n == 0
                    continue
                a_c = a_s[st:st + n]
                b_c = b_s[st:st + n]
                s_c = sid_s[st:st + n]
                st += n
                pa = np.full(want - n, (a_c[0] if n else CB[cc // NCHUNK]),
                             np.int64)
                pb = np.full(want - n, (b_c[0] if n else CB[cc % NCHUNK]),
                             np.int64)
                a_out.append(np.concatenate([a_c, pa]))
                b_out.append(np.concatenate([b_c, pb]))
                m = np.zeros((3, want), np.float32)
                m[s_c, np.arange(n)] = 1.0
                m_out.append(m)
            per_core.append((np.concatenate(a_out), np.concatenate(b_out),
                             np.concatenate(m_out, axis=1)))
        return cellB, per_core

    plan.setU = pack_union()
    plan.sU = int(plan.setU[0].sum())

    def set_gathers(cellB):
        a_g, b_g = [], []
        off = 0
        for ca in range(NCHUNK):
            row = cellB[ca * NCHUNK:(ca + 1) * NCHUNK]
            n_slots = int(row.sum())
            if n_slots:
                a_g.append(dict(c=ca, dst_slot=off, n_idx=n_slots * P))
            o2 = off
            for cb in range(NCHUNK):
                if cellB[ca * NCHUNK + cb]:
                    b_g.append(dict(c=cb, dst_slot=o2,
                                    n_idx=int(cellB[ca * NCHUNK + cb]) * P))
                    o2 += int(cellB[ca * NCHUNK + cb])
            off += n_slots
        return a_g, b_g

    plan.gU = set_gathers(plan.setU[0])

    # ---- build per-core input arrays ----
    x0 = np.concatenate([np.asarray(user_emb, np.float32),
                         np.asarray(item_emb, np.float32)])
    x0_p = np.zeros((NT, EMB), np.float32)
    x0_p[perm] = x0
    iota = np.tile(np.arange(P, dtype=np.float32)[None, :],
                   (P, 1)).astype(ml_dtypes.bfloat16)

    # per-core table rows in core-tile-major (stage) order
    t_all = np.repeat(np.arange(TILES), P)
    l_all = np.tile(np.arange(P), TILES)

    in_maps = []
    for i in range(NCORES):
        idx_blocks = []
        srows = np.zeros((P, plan.tot_slots), np.float32)
        svals = np.zeros((P, plan.tot_slots), np.float32)
        for g in plan.gathers:
            tl = sb_tiles[g["sb"]]
            c = g["c"]
            parts = []
            for t in tl:
                bslots = int(B[t, c])
                if bslots == 0:
                    continue
                cellk = (i * TILES + t) * NCHUNK + c
                st, en = starts[cellk], starts[cellk + 1]
                n = int(en - st)
                want = bslots * P
                lc = lcol_s[st:en]
                lr = lrow_s[st:en]
                vv = vals_s[st:en]
                pad = want - n
                lc = np.concatenate([lc, np.zeros(pad, np.int16)])
                lr = np.concatenate([lr, np.zeros(pad, np.float32)])
                vv = np.concatenate([vv, np.zeros(pad, np.float32)])
                parts.append(lc)
                so = slot_off[t, c]
                srows[:, so:so + bslots] = lr.reshape(bslots, P).T
                svals[:, so:so + bslots] = vv.reshape(bslots, P).T
            lc_all = np.concatenate(parts) if parts else np.zeros(0, np.int16)
            assert lc_all.size == g["n_idx"]
            idx_blocks.append(_wrap_idx(lc_all))

        def batch_idx(plan_set, gset):
            (a, b, m3) = plan_set[1][i]
            out = []
            for g in gset[0]:
                sl = a[g["dst_slot"] * P: g["dst_slot"] * P + g["n_idx"]]
                out.append(_wrap_idx((sl - CB[g["c"]]).astype(np.int16)))
            for g in gset[1]:
                sl = b[g["dst_slot"] * P: g["dst_slot"] * P + g["n_idx"]]
                out.append(_wrap_idx((sl - CB[g["c"]]).astype(np.int16)))
            return out, m3

        bU, mU = batch_idx(plan.setU, plan.gU)
        idx_blocks += bU
        idx_all = np.concatenate(idx_blocks, axis=1)   # [16, cols]
        masks = np.concatenate(
            [mU[k].reshape(-1, P).T for k in range(3)], axis=1
        ).astype(np.float32)

        own_rows = row_of(i, t_all, l_all)
        own0 = x0_p[own_rows]

        in_maps.append({
            "own0": own0.astype(ml_dtypes.float8_e4m3),
            "idxs": np.ascontiguousarray(idx_all),
            "srows": srows.astype(np.int8),
            "svals": svals.astype(ml_dtypes.float8_e4m3),
            "iota_in": iota,
            "lmask": masks,
        })

    plan.idx_cols_total = in_maps[0]["idxs"].shape[1]
    col = plan.idx_cols_edges
    plan.batch_cols = []
    for g in plan.gU[0] + plan.gU[1]:
        plan.batch_cols.append(col)
        col += g["n_idx"] // 16
    return plan, in_maps


# ============================================================
# Bass program
# ============================================================

NQUEUES = 4                      # SWDGE queues, round-robin for dma_gather


def build_nc(plan):
    nc = bacc.Bacc("TRN2", target_bir_lowering=False, debug=False,
                   num_devices=1 if DEBUG_SINGLE else NCORES,
                   num_swdge_queues=NQUEUES)
    f32 = mybir.dt.float32
    qrr = iter(range(1 << 30))  # round-robin counter for gather queues

    own0 = nc.dram_tensor("own0", [RPC, EMB], WDT, kind="ExternalInput")
    idxs = nc.dram_tensor("idxs", [16, plan.idx_cols_total], mybir.dt.int16,
                          kind="ExternalInput")
    srows_in = nc.dram_tensor("srows", [P, plan.tot_slots], mybir.dt.int8,
                              kind="ExternalInput")
    svals_in = nc.dram_tensor("svals", [P, plan.tot_slots], WDT,
                              kind="ExternalInput")
    iota_in = nc.dram_tensor("iota_in", [P, P], DT, kind="ExternalInput")
    lmask_in = nc.dram_tensor("lmask", [P, 3 * plan.sU], f32,
                              kind="ExternalInput")
    partials = nc.dram_tensor("partials", [1, 8], f32, kind="ExternalOutput")

    # wire (fp8, Shared) and local bf16 tables, one tensor per slice
    tab = [[nc.dram_tensor(f"tab{l}_{s}", [SLICE_ROWS[s], EMB], WDT,
                           addr_space="Shared")
            for s in range(NSLICE)] for l in range(N_LAYERS + 1)]
    tbl = [[nc.dram_tensor(f"tbl{l}_{s}", [SLICE_ROWS[s], EMB], DT)
            for s in range(NSLICE)] for l in range(N_LAYERS + 1)]
    # per-layer fp8 stage slices (AG inputs); layer 0's input is own0 itself
    stg = [[nc.dram_tensor(f"stg{l}_{s}", [NT_S[s] * P, EMB], WDT)
            for s in range(NSLICE)] for l in range(N_LAYERS)]

    groups = [list(range(NCORES))]

    def allgather(in_ap, out_ap):
        if DEBUG_SINGLE:
            rows = in_ap.shape[0]
            nc.sync.dma_start(out=out_ap[0:rows, :], in_=in_ap)
        else:
            nc.gpsimd.collective_compute(
                "AllGather", mybir.AluOpType.bypass,
                replica_groups=groups, ins=[in_ap], outs=[out_ap])

    with tile.TileContext(nc, num_cores=NCORES) as tc:
        with (
            tc.tile_pool(name="persist", bufs=1) as pers,
            tc.tile_pool(name="spool", bufs=24) as spool,
            tc.tile_pool(name="xpool", bufs=8) as xpool,
            tc.tile_pool(name="psum", bufs=4, space="PSUM") as pp,
        ):
            # ---- persistent loads ----
            idx_t = pers.tile([128, plan.idx_cols_total], mybir.dt.int16)
            srow_t = pers.tile([P, plan.tot_slots], f32)
            sval_t = pers.tile([P, plan.tot_slots], f32)
            iota_t = pers.tile([P, P], DT)
            mask_t = pers.tile([P, 3 * plan.sU], f32)
            ones_t = pers.tile([P, 1], f32)
            lane_t = pers.tile([P, 1], f32)
            id8_t = pers.tile([P, P], WDT)
            idb_t = pers.tile([P, P], DT)
            if not DEBUG_SKIP_PERSIST:
                for kk in range(8):
                    nc.sync.dma_start(out=idx_t[16 * kk:16 * (kk + 1), :],
                                      in_=idxs[:, :])
                srow8 = pers.tile([P, plan.tot_slots], mybir.dt.int8,
                                  tag="srow8")
                nc.sync.dma_start(out=srow8[:], in_=srows_in[:, :])
                nc.vector.tensor_copy(out=srow_t[:], in_=srow8[:])
                sval8 = pers.tile([P, plan.tot_slots], WDT, tag="sval8")
                nc.sync.dma_start(out=sval8[:], in_=svals_in[:, :])
                nc.vector.tensor_copy(out=sval_t[:], in_=sval8[:])
                nc.sync.dma_start(out=iota_t[:], in_=iota_in[:, :])
                nc.sync.dma_start(out=mask_t[:], in_=lmask_in[:, :])
                nc.gpsimd.memset(ones_t[:], 1.0)
                nc.gpsimd.iota(lane_t[:], pattern=[[0, 1]], base=0,
                               channel_multiplier=1,
                               allow_small_or_imprecise_dtypes=True)
                # identities: id[p, j] = (iota[p, j] == p)
                nc.vector.tensor_scalar(
                    out=idb_t[:], in0=iota_t[:], scalar1=lane_t[:],
                    scalar2=None, op0=mybir.AluOpType.is_equal)
                nc.vector.tensor_copy(out=id8_t[:], in_=idb_t[:])
                # initial table: AG own0 slices (via an internal staging
                # tensor — collectives cannot read IO tensors), upconvert
                # to bf16 local
                stg0 = nc.dram_tensor("stg0", [RPC, EMB], WDT)
                nc.sync.dma_start(out=stg0[:, :], in_=own0[:, :])
                for s in range(NSLICE):
                    allgather(stg0[T_BOUNDS[s] * P:T_BOUNDS[s + 1] * P, :],
                              tab[0][s][:, :])
                    nc.gpsimd.dma_start(out=tbl[0][s][:, :],
                                        in_=tab[0][s][:, :])

            # ---- N_LAYERS SpMM layers ----
            gpool_cm = tc.tile_pool(name="gpool", bufs=5)
            gpool = gpool_cm.__enter__()
            for layer in range(DEBUG_LAYERS):
                last = layer == N_LAYERS - 1
                sb_list = plan.sb_tiles if DEBUG_SB_LIMIT is None \
                    else plan.sb_tiles[:DEBUG_SB_LIMIT]
                ag_done = [False] * NSLICE
                for si, tl in enumerate(sb_list):
                    g_t = gpool.tile([P, plan.g_slots, EMB], DT, tag="G")
                    sb_base = plan.sb_slot_base[si]
                    if not DEBUG_COMPUTE_ONLY:
                        for g in plan.gathers:
                            if g["sb"] != si:
                                continue
                            c = g["c"]
                            src = tbl[layer][c]
                            for off in range(0, g["n_idx"], GMAX):
                                n = min(GMAX, g["n_idx"] - off)
                                nc.gpsimd.dma_gather(
                                    out_ap=g_t[:, g["dst_slot"] + off // P:
                                               g["dst_slot"] + (off + n) // P, :],
                                    in_ap=src[:, :],
                                    idxs_ap=idx_t[:, g["idx_col"] + off // 16:
                                                  g["idx_col"] + (off + n) // 16],
                                    num_idxs=n,
                                    num_idxs_reg=n,
                                    elem_size=EMB,
                                    queue_num=next(qrr) % NQUEUES,
                                )
                    if DEBUG_GATHER_ONLY:
                        continue
                    for t in tl:
                        s = int(_TILE_SLICE[t])
                        nslots = int(plan.B[t].sum())
                        ps = pp.tile([P, EMB], f32, tag="ps", space="PSUM")
                        k = 0
                        nmm = nslots + (N_LAYERS if last else 0)
                        if last:
                            # fold own0 + earlier layers' own rows (the local
                            # fp8 stage slices) via identity matmuls
                            o8 = xpool.tile([P, EMB], WDT, tag="O8")
                            nc.sync.dma_start(out=o8[:],
                                              in_=own0[t * P:(t + 1) * P, :])
                            nc.tensor.matmul(out=ps[:], lhsT=id8_t[:],
                                             rhs=o8[:], start=(k == 0),
                                             stop=(k == nmm - 1))
                            k += 1
                            lo = (t - T_BOUNDS[s]) * P
                            for ll in range(N_LAYERS - 1):
                                ob = xpool.tile([P, EMB], WDT, tag="OB")
                                nc.sync.dma_start(
                                    out=ob[:], in_=stg[ll][s][lo:lo + P, :])
                                nc.tensor.matmul(out=ps[:], lhsT=id8_t[:],
                                                 rhs=ob[:], start=False,
                                                 stop=(k == nmm - 1))
                                k += 1
                        for c in range(NCHUNK):
                            for j in range(int(plan.B[t, c])):
                                gs = plan.slot_off[t, c] + j
                                s_t = spool.tile([P, P], DT, tag="S")
                                nc.vector.tensor_scalar(
                                    out=s_t[:],
                                    in0=iota_t[:],
                                    scalar1=srow_t[:, gs, None],
                                    scalar2=sval_t[:, gs, None],
                                    op0=mybir.AluOpType.is_equal,
                                    op1=mybir.AluOpType.mult,
                                )
                                nc.tensor.matmul(
                                    out=ps[:],
                                    lhsT=s_t[:],
                                    rhs=g_t[:, gs - sb_base, :],
                                    start=(k == 0),
                                    stop=(k == nmm - 1),
                                )
                                k += 1
                        st_t = xpool.tile([P, EMB], WDT, tag="ST")
                        if nmm:
                            if last:
                                nc.scalar.mul(st_t[:], ps[:],
                                              1.0 / (N_LAYERS + 1))
                            else:
                                nc.scalar.activation(
                                    st_t[:], ps[:],
                                    mybir.ActivationFunctionType.Copy)
                        else:
                            nc.vector.memset(st_t[:], 0.0)
                        dst = stg[layer][s]
                        nc.sync.dma_start(
                            out=dst[(t - T_BOUNDS[s]) * P:
                                    (t - T_BOUNDS[s] + 1) * P, :],
                            in_=st_t[:])
                        # fire slice AG as soon as its last tile stored
                        if t == T_BOUNDS[s + 1] - 1:
                            ag_done[s] = True
                            allgather(stg[layer][s][:, :],
                                      tab[layer + 1][s][:, :])
                            nc.gpsimd.dma_start(out=tbl[layer + 1][s][:, :],
                                                in_=tab[layer + 1][s][:, :])
                if not DEBUG_GATHER_ONLY:
                    for s in range(NSLICE):   # partial-sb debug runs
                        if not ag_done[s] and DEBUG_SB_LIMIT is not None:
                            allgather(stg[layer][s][:, :],
                                      tab[layer + 1][s][:, :])
                            nc.gpsimd.dma_start(out=tbl[layer + 1][s][:, :],
                                                in_=tab[layer + 1][s][:, :])

            gpool_cm.__exit__(None, None, None)

            # ---- loss phase ----
            lpool_cm = tc.tile_pool(name="lpool", bufs=1)
            gpool = lpool_cm.__enter__()
            f32t = mybir.dt.float32
            if DEBUG_SKIP_LOSS:
                zz = pers.tile([1, 8], f32t)
                nc.vector.memset(zz[:], 0.0)
                nc.sync.dma_start(out=partials[:, :], in_=zz[:])
            else:
                part_t = pers.tile([P, 8], f32t)
                nc.vector.memset(part_t[:], 0.0)

                bcol = iter(plan.batch_cols)
                finalL = tbl[N_LAYERS]

                def gather_set(gset, nslots):
                    a_t = gpool.tile([P, max(nslots, 1), EMB], DT, tag="BA")
                    b_t = gpool.tile([P, max(nslots, 1), EMB], DT, tag="BB")
                    for dst, glist in ((a_t, gset[0]), (b_t, gset[1])):
                        for g in glist:
                            col = next(bcol)
                            c = g["c"]
                            for off in range(0, g["n_idx"], GMAX):
                                n = min(GMAX, g["n_idx"] - off)
                                nc.gpsimd.dma_gather(
                                    out_ap=dst[:, g["dst_slot"] + off // P:
                                               g["dst_slot"] + (off + n) // P, :],
                                    in_ap=finalL[c][:, :],
                                    idxs_ap=idx_t[:, col + off // 16:
                                                  col + (off + n) // 16],
                                    num_idxs=n,
                                    num_idxs_reg=n,
                                    elem_size=EMB,
                                    queue_num=next(qrr) % NQUEUES,
                                )
                    return a_t, b_t

                def masked_sum(x_t, m_ap, nslots, out_col):
                    tmp = spool.tile([P, nslots], f32t, tag="MS")
                    nc.vector.tensor_tensor(out=tmp[:], in0=x_t[:],
                                            in1=m_ap,
                                            op=mybir.AluOpType.mult)
                    nc.vector.tensor_reduce(out=part_t[:, out_col, None],
                                            in_=tmp[:],
                                            axis=mybir.AxisListType.X,
                                            op=mybir.AluOpType.add)

                sU = plan.sU
                a_t, b_t = gather_set(plan.gU, sU)
                prod = gpool.tile([P, sU, EMB], f32t, tag="PR")
                nc.vector.tensor_tensor(out=prod[:], in0=a_t[:, :sU, :],
                                        in1=b_t[:, :sU, :],
                                        op=mybir.AluOpType.mult)
                d_t = spool.tile([P, sU], f32t, tag="D")
                nc.vector.tensor_reduce(out=d_t[:], in_=prod[:],
                                        axis=mybir.AxisListType.X,
                                        op=mybir.AluOpType.add)
                pred = spool.tile([P, sU], f32t, tag="PRS")
                nc.scalar.activation(pred[:], d_t[:],
                                     mybir.ActivationFunctionType.Sigmoid)
                lnp = spool.tile([P, sU], f32t, tag="LNP")
                nc.scalar.activation(lnp[:], pred[:],
                                     mybir.ActivationFunctionType.Ln)
                ln1mp = spool.tile([P, sU], f32t, tag="LN1MP")
                nc.scalar.activation(ln1mp[:], pred[:],
                                     mybir.ActivationFunctionType.Ln,
                                     bias=1.0, scale=-1.0)
                plnp = spool.tile([P, sU], f32t, tag="PLNP")
                nc.vector.tensor_tensor(out=plnp[:], in0=pred[:], in1=lnp[:],
                                        op=mybir.AluOpType.mult)
                m0 = mask_t[:, 0 * sU:1 * sU]
                m1 = mask_t[:, 1 * sU:2 * sU]
                m2 = mask_t[:, 2 * sU:3 * sU]
                masked_sum(lnp, m0, sU, 0)     # q0 = sum ln(pred_pos)
                masked_sum(ln1mp, m1, sU, 1)   # q1 = sum ln(1-pred_neg)
                masked_sum(pred, m0, sU, 2)    # q2a = sum pred_pos
                masked_sum(plnp, m0, sU, 3)    # q3a = sum pred*ln(pred)
                masked_sum(pred, m1, sU, 4)    # q2b = sum pred_neg
                masked_sum(plnp, m1, sU, 5)    # q3b
                masked_sum(pred, m2, sU, 6)    # q4 = sum pred_ul

                pps = pp.tile([1, 8], f32t, tag="pps", space="PSUM")
                nc.tensor.matmul(out=pps[:], lhsT=ones_t[:], rhs=part_t[:],
                                 start=True, stop=True)
                res_t = pers.tile([1, 8], f32t)
                nc.scalar.activation(res_t[:], pps[:],
                                     mybir.ActivationFunctionType.Copy)
                nc.sync.dma_start(out=partials[:, :], in_=res_t[:])
            lpool_cm.__exit__(None, None, None)

    nc.compile()
    return nc


# ============================================================
# Public entry
# ============================================================

def host_combine(results):
    q = np.zeros(8, np.float64)
    for r in results:
        q += r["partials"].reshape(-1).astype(np.float64)
    B2 = 2.0 * BATCH
    bce = -(q[0] + q[1]) / B2
    pred_avg = (q[2] + q[4]) / B2
    pred_ul_avg = q[6] / B2
    gamma_term = (q[3] + q[5]) / B2
    info = ALPHA * (-pred_avg * np.log(pred_ul_avg)
                    - (1.0 - pred_avg) * np.log(1.0 - pred_ul_avg)) \
        + GAMMA * gamma_term
    return np.float32(bce), np.float32(info)


def kernel(**inputs):
    plan, in_maps = host_pack(**inputs)
    nc = build_nc(plan)
    res = run_bass_kernel_spmd(nc, in_maps, core_ids=list(range(NCORES)))
    return host_combine(res.results)


if __name__ == "__main__":
    pass
